# revision 25
# baseline (speedup 1.0000x reference)
"""Position-aware causal attention on 8 trn2 cores.

Sharding: data-parallel over batch (4) x tensor-parallel over heads (2 groups
of 8).  Core c handles batch c//2, heads (c%2)*8 .. +8.  Each core computes
qkv projection for its head slice, the full TxT attention for its heads with
position-causal masking (block-skipped using the sorted position structure),
and a partial output projection; the host sums the two head-group partials.
"""

import dataclasses

import numpy as np
import ml_dtypes

import concourse.bass as bass
import concourse.bacc as bacc
import concourse.tile as tile
from concourse import mybir
from concourse.bass_utils import run_bass_kernel_spmd

B, T, D, H, HD = 4, 2048, 1024, 16, 64
NC = 8
HPC = 8            # heads per core
NPAIR = HPC // 2   # head pairs per core
JT = T // 128      # 16 key tiles (PSUM partition dim)
IT = T // 512      # 4 query blocks (free dim)
KD = D // 128      # 8 contraction tiles for the projections

BF16 = mybir.dt.bfloat16
F32 = mybir.dt.float32
F32R = mybir.dt.float32r
NP_BF16 = ml_dtypes.bfloat16

_prog_cache = {}


def _schedule(position_ids):
    """needed[j][i], apply_mask[j][i] unions over all batches (SPMD: one
    program for all cores)."""
    pos = np.asarray(position_ids)
    needed = np.zeros((JT, IT), dtype=bool)
    full = np.ones((JT, IT), dtype=bool)  # fully unmasked for every batch
    for b in range(B):
        p = pos[b]
        minq = p[np.arange(IT) * 512]
        maxq = p[np.arange(IT) * 512 + 511]
        mink = p[np.arange(JT) * 128]
        maxk = p[np.arange(JT) * 128 + 127]
        nb = maxq[None, :] >= mink[:, None]      # any pair attends
        fb = minq[None, :] >= maxk[:, None]      # every pair attends
        needed |= nb
        full &= fb
    apply_mask = needed & ~full
    return needed, apply_mask


def _build(needed, apply_mask):
    nc = bacc.Bacc("TRN2", target_bir_lowering=False, debug=False)

    xT = nc.dram_tensor("xT", [D, T], BF16, kind="ExternalInput").ap()
    wqk = nc.dram_tensor("wqk", [D, D], BF16, kind="ExternalInput").ap()
    wv = nc.dram_tensor("wv", [D, D // 2], BF16, kind="ExternalInput").ap()
    wout = nc.dram_tensor("wout", [D // 2, D], BF16, kind="ExternalInput").ap()
    posq = nc.dram_tensor("posq", [1, T], F32, kind="ExternalInput").ap()
    posk = nc.dram_tensor("posk", [128, JT], F32, kind="ExternalInput").ap()
    out = nc.dram_tensor("out", [T, D], F32, kind="ExternalOutput").ap()

    with tile.TileContext(nc) as tc:
        with (
            tc.tile_pool(name="weights", bufs=1) as wpool,
            tc.tile_pool(name="qkT", bufs=1) as qkpool,
            tc.tile_pool(name="vp", bufs=1) as vpool,
            tc.tile_pool(name="attnT", bufs=1) as atpool,
            tc.tile_pool(name="masks", bufs=1) as mpool,
            tc.tile_pool(name="small", bufs=4) as spool,
            tc.tile_pool(name="etile", bufs=3) as epool,
            tc.tile_pool(name="ostage", bufs=3) as opool,
            tc.tile_pool(name="psB", bufs=2, space="PSUM") as psB,
            tc.tile_pool(name="psS", bufs=2, space="PSUM") as psS,
            tc.tile_pool(name="psPV", bufs=1, space="PSUM") as psPV,
        ):
            # ---- input loads -------------------------------------------------
            xT_sb = []
            wqk_sb = []
            wv_sb = []
            for k in range(KD):
                t = wpool.tile([128, T], BF16, tag=f"xT{k}")
                for n in range(IT):
                    nc.gpsimd.dma_start(
                        t[:, n * 512:(n + 1) * 512],
                        xT[k * 128:(k + 1) * 128, n * 512:(n + 1) * 512],
                    )
                xT_sb.append(t)
            for k in range(KD):
                t = wpool.tile([128, D], BF16, tag=f"wqk{k}")
                nc.gpsimd.dma_start(t[:], wqk[k * 128:(k + 1) * 128, :])
                wqk_sb.append(t)
            for k in range(KD):
                t = wpool.tile([128, D // 2], BF16, tag=f"wv{k}")
                nc.gpsimd.dma_start(t[:], wv[k * 128:(k + 1) * 128, :])
                wv_sb.append(t)
            wout_sb = []
            for k in range(4):
                t = wpool.tile([128, D], BF16, tag=f"wout{k}")
                nc.gpsimd.dma_start(t[:], wout[k * 128:(k + 1) * 128, :])
                wout_sb.append(t)
            posk_sb = wpool.tile([128, JT], F32, tag="posk")
            nc.gpsimd.dma_start(posk_sb[:], posk[:, :])
            ones_f32 = wpool.tile([1, 64], F32, tag="ones_f32")
            nc.vector.memset(ones_f32[:], 1.0)
            ones_sb = wpool.tile([1, 64], F32R, tag="ones")
            nc.vector.tensor_copy(ones_sb[:], ones_f32[:])
            # broadcast posq to 128 partitions, one tile per i-block
            pbq_sb = []
            for i in range(IT):
                t = wpool.tile([128, 512], F32, tag=f"pbq{i}")
                src = bass.AP(
                    tensor=posq.tensor,
                    offset=posq.offset + i * 512,
                    ap=[[0, 128], [1, 512]],
                )
                nc.gpsimd.dma_start(t[:], src)
                pbq_sb.append(t)

            # ---- qk projection: qkT[m] = (Wqk[:, m-block]).T @ xT ------------
            # m order interleaves q/k so pair p's attention unblocks early.
            qkT_sb = [None] * KD
            for m in [0, 4, 1, 5, 2, 6, 3, 7]:
                dst = qkpool.tile([128, T], BF16, tag=f"qkT{m}")
                qkT_sb[m] = dst
                for n in range(IT):
                    ps = psB.tile([128, 512], F32)
                    for k in range(KD):
                        nc.tensor.matmul(
                            ps[:],
                            lhsT=wqk_sb[k][:, m * 128:(m + 1) * 128],
                            rhs=xT_sb[k][:, n * 512:(n + 1) * 512],
                            start=(k == 0),
                            stop=(k == KD - 1),
                        )
                    nc.vector.tensor_copy(dst[:, n * 512:(n + 1) * 512], ps[:])

            # ---- v projection into [v_h | 1] packed layout -------------------
            vp_sb = []
            for jt in range(JT):
                dst = vpool.tile([128, HPC, 65], BF16, tag=f"vp{jt}")
                nc.vector.memset(dst[:, :, 64:65], 1.0)
                ps = psB.tile([128, HPC, 64], F32, tag="ps")
                for k in range(KD):
                    nc.tensor.matmul(
                        ps[:],
                        lhsT=xT_sb[k][:, jt * 128:(jt + 1) * 128],
                        rhs=wv_sb[k][:],
                        start=(k == 0),
                        stop=(k == KD - 1),
                    )
                nc.vector.tensor_copy(dst[:, :, 0:64], ps[:])
                vp_sb.append(dst)

            # ---- position masks for partial blocks (shared by all heads) ----
            U_sb = {}
            for j in range(JT):
                for i in range(IT):
                    if apply_mask[j][i]:
                        u = mpool.tile([128, 512], BF16, tag=f"U{j}_{i}")
                        nc.vector.tensor_scalar(
                            out=u[:],
                            in0=pbq_sb[i][:],
                            scalar1=posk_sb[:, j:j + 1],
                            scalar2=None,
                            op0=mybir.AluOpType.is_ge,
                        )
                        U_sb[(j, i)] = u

            # ---- attention ---------------------------------------------------
            attnT_sb = []
            for p in range(NPAIR):
                at_t = atpool.tile([128, T], BF16, tag=f"attnT{p}")
                attnT_sb.append(at_t)

            def epilogue(p, i, ppsb, dcp, h2):
                """recip -> PE broadcast -> normalize into attnT (deferred one
                group so the PE queue isn't blocked behind the DVE chain)."""
                rt = spool.tile([1, 512], F32, tag="recip")
                nc.vector.reciprocal_approx_fast(rt[:], dcp[:])
                rtr = spool.tile([1, 512], F32R, tag="recipr")
                nc.vector.tensor_copy(rtr[:], rt[:])
                bt = psB.tile([128, 512], F32, tag="ps")
                nc.tensor.matmul(
                    bt[0:64, :],
                    lhsT=ones_sb[0:1, :],
                    rhs=rtr[:],
                    start=True, stop=True,
                )
                nc.vector.tensor_mul(
                    attnT_sb[p][h2 * 64:(h2 + 1) * 64, i * 512:(i + 1) * 512],
                    ppsb[:],
                    bt[0:64, :],
                )

            pending = []
            for p in range(NPAIR):
                kT = qkT_sb[NPAIR + p]
                qT = qkT_sb[p]
                for i in range(IT):
                    js = [j for j in range(JT) if needed[j][i]]
                    ppa = psPV.tile([65, 512], F32, tag="ppa")
                    ppb = psPV.tile([65, 512], F32, tag="ppb")
                    for idx, j in enumerate(js):
                        st = psS.tile([128, 1024], F32)
                        nc.tensor.matmul(
                            st[:, 0:512],
                            lhsT=kT[0:64, j * 128:(j + 1) * 128],
                            rhs=qT[0:64, i * 512:(i + 1) * 512],
                            start=True, stop=True,
                        )
                        nc.tensor.matmul(
                            st[:, 512:1024],
                            lhsT=kT[64:128, j * 128:(j + 1) * 128],
                            rhs=qT[64:128, i * 512:(i + 1) * 512],
                            start=True, stop=True,
                        )
                        et = epool.tile([128, 1024], BF16)
                        nc.scalar.activation(
                            et[:], st[:], mybir.ActivationFunctionType.Exp
                        )
                        if apply_mask[j][i]:
                            u = U_sb[(j, i)]
                            uap = u[:]
                            urep = dataclasses.replace(
                                uap, ap=[uap.ap[0], [0, 2], uap.ap[1]]
                            )
                            et3 = et[:].rearrange("q (a b) -> q a b", a=2)
                            nc.vector.tensor_mul(et3, et3, urep)
                        first = idx == 0
                        last = idx == len(js) - 1
                        nc.tensor.matmul(
                            ppa[:],
                            lhsT=vp_sb[j][:, 2 * p, :],
                            rhs=et[:, 0:512],
                            start=first, stop=last,
                        )
                        nc.tensor.matmul(
                            ppb[:],
                            lhsT=vp_sb[j][:, 2 * p + 1, :],
                            rhs=et[:, 512:1024],
                            start=first, stop=last,
                        )
                    # free the PV banks fast: copy out^T and denom to SBUF,
                    # defer the normalize chain by one group.
                    for h2, pp in ((0, ppa), (1, ppb)):
                        ppsb = spool.tile([64, 512], F32, tag="ppsb")
                        nc.vector.tensor_copy(ppsb[:], pp[0:64, :])
                        dcp = spool.tile([1, 512], F32, tag="dcp")
                        nc.vector.tensor_copy(dcp[:], pp[64:65, :])
                        pending.append((p, i, ppsb, dcp, h2))
                    while len(pending) > 2:
                        epilogue(*pending.pop(0))
            for args in pending:
                epilogue(*args)

            # ---- output projection ------------------------------------------
            for mt in range(JT):
                for n in range(2):
                    ps = psB.tile([128, 512], F32)
                    for kp in range(NPAIR):
                        nc.tensor.matmul(
                            ps[:],
                            lhsT=attnT_sb[kp][:, mt * 128:(mt + 1) * 128],
                            rhs=wout_sb[kp][:, n * 512:(n + 1) * 512],
                            start=(kp == 0),
                            stop=(kp == NPAIR - 1),
                        )
                    ot = opool.tile([128, 512], F32, tag="ot")
                    nc.scalar.copy(ot[:], ps[:])
                    nc.gpsimd.dma_start(
                        out[mt * 128:(mt + 1) * 128, n * 512:(n + 1) * 512],
                        ot[:],
                    )

    nc.compile()
    return nc


def kernel(x, position_ids, Wqkv, Wout, _trace=False, _trace_kwargs=None):
    x = np.asarray(x, dtype=np.float32)
    pos = np.asarray(position_ids)
    Wqkv = np.asarray(Wqkv, dtype=np.float32)
    Wout = np.asarray(Wout, dtype=np.float32)

    needed, apply_mask = _schedule(pos)
    key = (needed.tobytes(), apply_mask.tobytes())
    if key not in _prog_cache:
        _prog_cache.clear()
        _prog_cache[key] = _build(needed, apply_mask)
    nc = _prog_cache[key]

    scale = 1.0 / np.sqrt(HD)
    in_maps = []
    xT_b = {}
    for c in range(NC):
        b, g = c // 2, c % 2
        if b not in xT_b:
            xT_b[b] = np.ascontiguousarray(x[b].T).astype(NP_BF16)
        heads = range(g * HPC, (g + 1) * HPC)
        wqk_c = np.concatenate(
            [Wqkv[:, h * HD:(h + 1) * HD] * scale for h in heads]
            + [Wqkv[:, D + h * HD:D + (h + 1) * HD] for h in heads],
            axis=1,
        ).astype(NP_BF16)
        wv_c = np.concatenate(
            [Wqkv[:, 2 * D + h * HD:2 * D + (h + 1) * HD] for h in heads], axis=1
        ).astype(NP_BF16)
        wout_c = np.concatenate(
            [Wout[h * HD:(h + 1) * HD, :] for h in heads], axis=0
        ).astype(NP_BF16)
        pf = pos[b].astype(np.float32)
        in_maps.append({
            "xT": xT_b[b],
            "wqk": np.ascontiguousarray(wqk_c),
            "wv": np.ascontiguousarray(wv_c),
            "wout": np.ascontiguousarray(wout_c),
            "posq": pf.reshape(1, T),
            "posk": np.ascontiguousarray(pf.reshape(JT, 128).T),
        })

    kw = dict(_trace_kwargs or {})
    res = run_bass_kernel_spmd(
        nc, in_maps, core_ids=list(range(NC)), trace=_trace, **kw
    )
    kernel.last_result = res
    out = np.empty((B, T, D), dtype=np.float32)
    for b in range(B):
        out[b] = res.results[2 * b]["out"] + res.results[2 * b + 1]["out"]
    return out


# revision 33
# speedup vs baseline: 1.0318x; 1.0318x over previous
"""Position-aware causal attention on 8 trn2 cores.

Sharding: data-parallel over batch (4) x tensor-parallel over heads (2 groups
of 8).  Core c handles batch c//2, heads (c%2)*8 .. +8.  Each core computes
qkv projection for its head slice, the full TxT attention for its heads with
position-causal masking (block-skipped using the sorted position structure),
and a partial output projection; the host sums the two head-group partials.
"""

import dataclasses

import numpy as np
import ml_dtypes

import concourse.bass as bass
import concourse.bacc as bacc
import concourse.tile as tile
from concourse import mybir
from concourse.bass_utils import run_bass_kernel_spmd

B, T, D, H, HD = 4, 2048, 1024, 16, 64
NC = 8
HPC = 8            # heads per core
NPAIR = HPC // 2   # head pairs per core
JT = T // 128      # 16 key tiles (PSUM partition dim)
IT = T // 512      # 4 query blocks (free dim)
KD = D // 128      # 8 contraction tiles for the projections

BF16 = mybir.dt.bfloat16
F32 = mybir.dt.float32
F32R = mybir.dt.float32r
NP_BF16 = ml_dtypes.bfloat16

_prog_cache = {}


def _schedule(position_ids):
    """needed[j][i], apply_mask[j][i] unions over all batches (SPMD: one
    program for all cores)."""
    pos = np.asarray(position_ids)
    needed = np.zeros((JT, IT), dtype=bool)
    full = np.ones((JT, IT), dtype=bool)  # fully unmasked for every batch
    for b in range(B):
        p = pos[b]
        minq = p[np.arange(IT) * 512]
        maxq = p[np.arange(IT) * 512 + 511]
        mink = p[np.arange(JT) * 128]
        maxk = p[np.arange(JT) * 128 + 127]
        nb = maxq[None, :] >= mink[:, None]      # any pair attends
        fb = minq[None, :] >= maxk[:, None]      # every pair attends
        needed |= nb
        full &= fb
    apply_mask = needed & ~full
    return needed, apply_mask


def _build(needed, apply_mask):
    nc = bacc.Bacc("TRN2", target_bir_lowering=False, debug=False)

    xT = nc.dram_tensor("xT", [D, T], BF16, kind="ExternalInput").ap()
    wqk = nc.dram_tensor("wqk", [D, D], BF16, kind="ExternalInput").ap()
    wv = nc.dram_tensor("wv", [D, D // 2], BF16, kind="ExternalInput").ap()
    wout = nc.dram_tensor("wout", [D // 2, D], BF16, kind="ExternalInput").ap()
    posq = nc.dram_tensor("posq", [1, T], F32, kind="ExternalInput").ap()
    posk = nc.dram_tensor("posk", [128, JT], F32, kind="ExternalInput").ap()
    out = nc.dram_tensor("out", [T, D], BF16, kind="ExternalOutput").ap()

    with tile.TileContext(nc) as tc:
        with (
            tc.tile_pool(name="weights", bufs=1) as wpool,
            tc.tile_pool(name="qkT", bufs=1) as qkpool,
            tc.tile_pool(name="vp", bufs=1) as vpool,
            tc.tile_pool(name="attnT", bufs=1) as atpool,
            tc.tile_pool(name="masks", bufs=1) as mpool,
            tc.tile_pool(name="small", bufs=4) as spool,
            tc.tile_pool(name="etile", bufs=3) as epool,
            tc.tile_pool(name="ostage", bufs=3) as opool,
            tc.tile_pool(name="psB", bufs=2, space="PSUM") as psB,
            tc.tile_pool(name="psS", bufs=2, space="PSUM") as psS,
            tc.tile_pool(name="psPV", bufs=1, space="PSUM") as psPV,
        ):
            # ---- input loads (xT+wv first: the v projection runs first) -----
            xT_sb = []
            wqk_sb = []
            wv_sb = []
            for k in range(KD):
                t = wpool.tile([128, T], BF16, tag=f"xT{k}")
                for n in range(IT):
                    nc.gpsimd.dma_start(
                        t[:, n * 512:(n + 1) * 512],
                        xT[k * 128:(k + 1) * 128, n * 512:(n + 1) * 512],
                    )
                xT_sb.append(t)
                t2 = wpool.tile([128, D // 2], BF16, tag=f"wv{k}")
                nc.gpsimd.dma_start(t2[:], wv[k * 128:(k + 1) * 128, :])
                wv_sb.append(t2)
            for k in range(KD):
                t = wpool.tile([128, D], BF16, tag=f"wqk{k}")
                nc.gpsimd.dma_start(t[:], wqk[k * 128:(k + 1) * 128, :])
                wqk_sb.append(t)
            posk_sb = wpool.tile([128, JT], F32, tag="posk")
            nc.gpsimd.dma_start(posk_sb[:], posk[:, :])
            ones_f32 = wpool.tile([1, 64], F32, tag="ones_f32")
            nc.vector.memset(ones_f32[:], 1.0)
            ones_sb = wpool.tile([1, 64], F32R, tag="ones")
            nc.vector.tensor_copy(ones_sb[:], ones_f32[:])
            # broadcast posq to 128 partitions, one tile per i-block
            pbq_sb = []
            for i in range(IT):
                t = wpool.tile([128, 512], F32, tag=f"pbq{i}")
                src = bass.AP(
                    tensor=posq.tensor,
                    offset=posq.offset + i * 512,
                    ap=[[0, 128], [1, 512]],
                )
                nc.gpsimd.dma_start(t[:], src)
                pbq_sb.append(t)
            wout_sb = []
            for k in range(4):
                t = wpool.tile([128, D], BF16, tag=f"wout{k}")
                nc.gpsimd.dma_start(t[:], wout[k * 128:(k + 1) * 128, :])
                wout_sb.append(t)

            # ---- position masks for partial blocks (shared by all heads) ----
            U_sb = {}
            for j in range(JT):
                for i in range(IT):
                    if apply_mask[j][i]:
                        u = mpool.tile([128, 512], BF16, tag=f"U{j}_{i}")
                        nc.vector.tensor_scalar(
                            out=u[:],
                            in0=pbq_sb[i][:],
                            scalar1=posk_sb[:, j:j + 1],
                            scalar2=None,
                            op0=mybir.AluOpType.is_ge,
                        )
                        U_sb[(j, i)] = u

            # ---- v projection into [v_h | 1] packed layout -------------------
            vp_sb = []
            for jt in range(JT):
                dst = vpool.tile([128, HPC, 65], BF16, tag=f"vp{jt}")
                nc.vector.memset(dst[:, :, 64:65], 1.0)
                ps = psB.tile([128, HPC, 64], F32, tag="ps")
                for k in range(KD):
                    nc.tensor.matmul(
                        ps[:],
                        lhsT=xT_sb[k][:, jt * 128:(jt + 1) * 128],
                        rhs=wv_sb[k][:],
                        start=(k == 0),
                        stop=(k == KD - 1),
                    )
                nc.vector.tensor_copy(dst[:, :, 0:64], ps[:])
                vp_sb.append(dst)

            # ---- qk projection, emitted chain-by-chain interleaved with the
            # previous pair's attention so the in-order PE queue fills the
            # slack of the ACT(exp)-paced attention groups ----
            qkT_sb = [None] * KD

            def proj_chain(m, n):
                dst = qkT_sb[m]
                ps = psB.tile([128, 512], F32, tag="ps")
                for k in range(KD):
                    nc.tensor.matmul(
                        ps[:],
                        lhsT=wqk_sb[k][:, m * 128:(m + 1) * 128],
                        rhs=xT_sb[k][:, n * 512:(n + 1) * 512],
                        start=(k == 0),
                        stop=(k == KD - 1),
                    )
                nc.vector.tensor_copy(dst[:, n * 512:(n + 1) * 512], ps[:])

            for m in range(KD):
                qk_t = qkpool.tile([128, T], BF16, tag=f"qkT{m}")
                qkT_sb[m] = qk_t

            # ---- attention ---------------------------------------------------
            attnT_sb = []
            for p in range(NPAIR):
                at_t = atpool.tile([128, T], BF16, tag=f"attnT{p}")
                attnT_sb.append(at_t)

            def epilogue(p, i, ppsb, dcp, h2):
                """recip -> PE broadcast -> normalize into attnT (deferred one
                group so the PE queue isn't blocked behind the DVE chain)."""
                rt = spool.tile([1, 512], F32, tag="recip")
                nc.vector.reciprocal_approx_fast(rt[:], dcp[:])
                rtr = spool.tile([1, 512], F32R, tag="recipr")
                nc.vector.tensor_copy(rtr[:], rt[:])
                bt = psB.tile([128, 512], F32, tag="ps")
                nc.tensor.matmul(
                    bt[0:64, :],
                    lhsT=ones_sb[0:1, :],
                    rhs=rtr[:],
                    start=True, stop=True,
                )
                nc.vector.tensor_mul(
                    attnT_sb[p][h2 * 64:(h2 + 1) * 64, i * 512:(i + 1) * 512],
                    ppsb[:],
                    bt[0:64, :],
                )

            pending = []
            # pair 0's projection runs up front; pair p+1's 8 chains are
            # injected two per attention i-group of pair p.
            for n in range(IT):
                proj_chain(0, n)
                proj_chain(NPAIR, n)
            for p in range(NPAIR):
                inject = []
                if p + 1 < NPAIR:
                    inject = [(p + 1, n) for n in range(IT)] + [
                        (NPAIR + p + 1, n) for n in range(IT)
                    ]
                kT = qkT_sb[NPAIR + p]
                qT = qkT_sb[p]
                for i in range(IT):
                    js = [j for j in range(JT) if needed[j][i]]
                    ppa = psPV.tile([65, 512], F32, tag="ppa")
                    ppb = psPV.tile([65, 512], F32, tag="ppb")
                    for idx, j in enumerate(js):
                        st = psS.tile([128, 1024], F32)
                        nc.tensor.matmul(
                            st[:, 0:512],
                            lhsT=kT[0:64, j * 128:(j + 1) * 128],
                            rhs=qT[0:64, i * 512:(i + 1) * 512],
                            start=True, stop=True,
                        )
                        nc.tensor.matmul(
                            st[:, 512:1024],
                            lhsT=kT[64:128, j * 128:(j + 1) * 128],
                            rhs=qT[64:128, i * 512:(i + 1) * 512],
                            start=True, stop=True,
                        )
                        et = epool.tile([128, 1024], BF16)
                        nc.scalar.activation(
                            et[:], st[:], mybir.ActivationFunctionType.Exp
                        )
                        if apply_mask[j][i]:
                            u = U_sb[(j, i)]
                            uap = u[:]
                            urep = dataclasses.replace(
                                uap, ap=[uap.ap[0], [0, 2], uap.ap[1]]
                            )
                            et3 = et[:].rearrange("q (a b) -> q a b", a=2)
                            nc.vector.tensor_mul(et3, et3, urep)
                        first = idx == 0
                        last = idx == len(js) - 1
                        nc.tensor.matmul(
                            ppa[:],
                            lhsT=vp_sb[j][:, 2 * p, :],
                            rhs=et[:, 0:512],
                            start=first, stop=last,
                        )
                        nc.tensor.matmul(
                            ppb[:],
                            lhsT=vp_sb[j][:, 2 * p + 1, :],
                            rhs=et[:, 512:1024],
                            start=first, stop=last,
                        )
                    # free the PV banks fast: copy out^T and denom to SBUF,
                    # defer the normalize chain by one group.
                    for h2, pp in ((0, ppa), (1, ppb)):
                        ppsb = spool.tile([64, 512], F32, tag="ppsb")
                        nc.vector.tensor_copy(ppsb[:], pp[0:64, :])
                        dcp = spool.tile([1, 512], F32, tag="dcp")
                        nc.vector.tensor_copy(dcp[:], pp[64:65, :])
                        pending.append((p, i, ppsb, dcp, h2))
                    while len(pending) > 2:
                        epilogue(*pending.pop(0))
                    # inject next pair's projection chains into PE slack
                    for _ in range(2):
                        if inject:
                            proj_chain(*inject.pop(0))
            for args in pending:
                epilogue(*args)

            # ---- output projection ------------------------------------------
            for mt in range(JT):
                for n in range(2):
                    ps = psB.tile([128, 512], F32)
                    for kp in range(NPAIR):
                        nc.tensor.matmul(
                            ps[:],
                            lhsT=attnT_sb[kp][:, mt * 128:(mt + 1) * 128],
                            rhs=wout_sb[kp][:, n * 512:(n + 1) * 512],
                            start=(kp == 0),
                            stop=(kp == NPAIR - 1),
                        )
                    ot = opool.tile([128, 512], BF16, tag="ot")
                    nc.scalar.copy(ot[:], ps[:])
                    nc.gpsimd.dma_start(
                        out[mt * 128:(mt + 1) * 128, n * 512:(n + 1) * 512],
                        ot[:],
                    )

    nc.compile()
    return nc


def kernel(x, position_ids, Wqkv, Wout, _trace=False, _trace_kwargs=None):
    x = np.asarray(x, dtype=np.float32)
    pos = np.asarray(position_ids)
    Wqkv = np.asarray(Wqkv, dtype=np.float32)
    Wout = np.asarray(Wout, dtype=np.float32)

    needed, apply_mask = _schedule(pos)
    key = (needed.tobytes(), apply_mask.tobytes())
    if key not in _prog_cache:
        _prog_cache.clear()
        _prog_cache[key] = _build(needed, apply_mask)
    nc = _prog_cache[key]

    scale = 1.0 / np.sqrt(HD)
    in_maps = []
    xT_b = {}
    for c in range(NC):
        b, g = c // 2, c % 2
        if b not in xT_b:
            xT_b[b] = np.ascontiguousarray(x[b].T).astype(NP_BF16)
        heads = range(g * HPC, (g + 1) * HPC)
        wqk_c = np.concatenate(
            [Wqkv[:, h * HD:(h + 1) * HD] * scale for h in heads]
            + [Wqkv[:, D + h * HD:D + (h + 1) * HD] for h in heads],
            axis=1,
        ).astype(NP_BF16)
        wv_c = np.concatenate(
            [Wqkv[:, 2 * D + h * HD:2 * D + (h + 1) * HD] for h in heads], axis=1
        ).astype(NP_BF16)
        wout_c = np.concatenate(
            [Wout[h * HD:(h + 1) * HD, :] for h in heads], axis=0
        ).astype(NP_BF16)
        pf = pos[b].astype(np.float32)
        in_maps.append({
            "xT": xT_b[b],
            "wqk": np.ascontiguousarray(wqk_c),
            "wv": np.ascontiguousarray(wv_c),
            "wout": np.ascontiguousarray(wout_c),
            "posq": pf.reshape(1, T),
            "posk": np.ascontiguousarray(pf.reshape(JT, 128).T),
        })

    kw = dict(_trace_kwargs or {})
    res = run_bass_kernel_spmd(
        nc, in_maps, core_ids=list(range(NC)), trace=_trace, **kw
    )
    kernel.last_result = res
    out = np.empty((B, T, D), dtype=np.float32)
    for b in range(B):
        out[b] = res.results[2 * b]["out"].astype(np.float32)
        out[b] += res.results[2 * b + 1]["out"].astype(np.float32)
    return out


# revision 35
# speedup vs baseline: 1.0729x; 1.0399x over previous
"""Position-aware causal attention on 8 trn2 cores.

Sharding: data-parallel over batch (4) x tensor-parallel over heads (2 groups
of 8).  Core c handles batch c//2, heads (c%2)*8 .. +8.  Each core computes
qkv projection for its head slice, the full TxT attention for its heads with
position-causal masking (block-skipped using the sorted position structure),
and a partial output projection; the host sums the two head-group partials.
"""

import dataclasses

import numpy as np
import ml_dtypes

import concourse.bass as bass
import concourse.bacc as bacc
import concourse.tile as tile
from concourse import mybir
from concourse.bass_utils import run_bass_kernel_spmd

B, T, D, H, HD = 4, 2048, 1024, 16, 64
NC = 8
HPC = 8            # heads per core
NPAIR = HPC // 2   # head pairs per core
JT = T // 128      # 16 key tiles (PSUM partition dim)
IT = T // 512      # 4 query blocks (free dim)
KD = D // 128      # 8 contraction tiles for the projections

BF16 = mybir.dt.bfloat16
F32 = mybir.dt.float32
F32R = mybir.dt.float32r
NP_BF16 = ml_dtypes.bfloat16

_prog_cache = {}


def _schedule(position_ids):
    """needed[j][i], apply_mask[j][i] unions over all batches (SPMD: one
    program for all cores)."""
    pos = np.asarray(position_ids)
    needed = np.zeros((JT, IT), dtype=bool)
    full = np.ones((JT, IT), dtype=bool)  # fully unmasked for every batch
    for b in range(B):
        p = pos[b]
        minq = p[np.arange(IT) * 512]
        maxq = p[np.arange(IT) * 512 + 511]
        mink = p[np.arange(JT) * 128]
        maxk = p[np.arange(JT) * 128 + 127]
        nb = maxq[None, :] >= mink[:, None]      # any pair attends
        fb = minq[None, :] >= maxk[:, None]      # every pair attends
        needed |= nb
        full &= fb
    apply_mask = needed & ~full
    return needed, apply_mask


def _build(needed, apply_mask):
    nc = bacc.Bacc("TRN2", target_bir_lowering=False, debug=False)

    xT = nc.dram_tensor("xT", [D, T], BF16, kind="ExternalInput").ap()
    wqk = nc.dram_tensor("wqk", [D, D], BF16, kind="ExternalInput").ap()
    wv = nc.dram_tensor("wv", [D, D // 2], BF16, kind="ExternalInput").ap()
    wout = nc.dram_tensor("wout", [D // 2, D], BF16, kind="ExternalInput").ap()
    posq = nc.dram_tensor("posq", [1, T], F32, kind="ExternalInput").ap()
    posk = nc.dram_tensor("posk", [128, JT], F32, kind="ExternalInput").ap()
    out = nc.dram_tensor("out", [T, D], BF16, kind="ExternalOutput").ap()

    with tile.TileContext(nc) as tc:
        with (
            tc.tile_pool(name="weights", bufs=1) as wpool,
            tc.tile_pool(name="qkT", bufs=1) as qkpool,
            tc.tile_pool(name="vp", bufs=1) as vpool,
            tc.tile_pool(name="attnT", bufs=1) as atpool,
            tc.tile_pool(name="masks", bufs=1) as mpool,
            tc.tile_pool(name="small", bufs=4) as spool,
            tc.tile_pool(name="etile", bufs=3) as epool,
            tc.tile_pool(name="ostage", bufs=3) as opool,
            tc.tile_pool(name="psB", bufs=2, space="PSUM") as psB,
            tc.tile_pool(name="psS", bufs=2, space="PSUM") as psS,
            tc.tile_pool(name="psPV", bufs=1, space="PSUM") as psPV,
        ):
            # ---- input loads (xT+wv first: the v projection runs first) -----
            xT_sb = []
            wqk_sb = []
            wv_sb = []
            for k in range(KD):
                t = wpool.tile([128, T], BF16, tag=f"xT{k}")
                for n in range(IT):
                    nc.gpsimd.dma_start(
                        t[:, n * 512:(n + 1) * 512],
                        xT[k * 128:(k + 1) * 128, n * 512:(n + 1) * 512],
                    )
                xT_sb.append(t)
                t2 = wpool.tile([128, D // 2], BF16, tag=f"wv{k}")
                nc.gpsimd.dma_start(t2[:], wv[k * 128:(k + 1) * 128, :])
                wv_sb.append(t2)
            for k in range(KD):
                t = wpool.tile([128, D], BF16, tag=f"wqk{k}")
                nc.gpsimd.dma_start(t[:], wqk[k * 128:(k + 1) * 128, :])
                wqk_sb.append(t)
            posk_sb = wpool.tile([128, JT], F32, tag="posk")
            nc.gpsimd.dma_start(posk_sb[:], posk[:, :])
            ones_f32 = wpool.tile([1, 64], F32, tag="ones_f32")
            nc.vector.memset(ones_f32[:], 1.0)
            ones_sb = wpool.tile([1, 64], F32R, tag="ones")
            nc.vector.tensor_copy(ones_sb[:], ones_f32[:])
            # broadcast posq to 128 partitions, one tile per i-block
            pbq_sb = []
            for i in range(IT):
                t = wpool.tile([128, 512], F32, tag=f"pbq{i}")
                src = bass.AP(
                    tensor=posq.tensor,
                    offset=posq.offset + i * 512,
                    ap=[[0, 128], [1, 512]],
                )
                nc.gpsimd.dma_start(t[:], src)
                pbq_sb.append(t)
            wout_sb = []
            for k in range(4):
                t = wpool.tile([128, D], BF16, tag=f"wout{k}")
                nc.gpsimd.dma_start(t[:], wout[k * 128:(k + 1) * 128, :])
                wout_sb.append(t)

            # ---- position masks for partial blocks (shared by all heads) ----
            U_sb = {}
            for j in range(JT):
                for i in range(IT):
                    if apply_mask[j][i]:
                        u = mpool.tile([128, 512], BF16, tag=f"U{j}_{i}")
                        nc.vector.tensor_scalar(
                            out=u[:],
                            in0=pbq_sb[i][:],
                            scalar1=posk_sb[:, j:j + 1],
                            scalar2=None,
                            op0=mybir.AluOpType.is_ge,
                        )
                        U_sb[(j, i)] = u

            # ---- v projection into [v_h | 1] packed layout -------------------
            vp_sb = []
            for jt in range(JT):
                dst = vpool.tile([128, HPC, 65], BF16, tag=f"vp{jt}")
                nc.vector.memset(dst[:, :, 64:65], 1.0)
                ps = psB.tile([128, HPC, 64], F32, tag="ps")
                for k in range(KD):
                    nc.tensor.matmul(
                        ps[:],
                        lhsT=xT_sb[k][:, jt * 128:(jt + 1) * 128],
                        rhs=wv_sb[k][:],
                        start=(k == 0),
                        stop=(k == KD - 1),
                    )
                nc.vector.tensor_copy(dst[:, :, 0:64], ps[:])
                vp_sb.append(dst)

            # ---- qk projection, emitted chain-by-chain interleaved with the
            # previous pair's attention so the in-order PE queue fills the
            # slack of the ACT(exp)-paced attention groups ----
            qkT_sb = [None] * KD

            def proj_chain(m, n):
                dst = qkT_sb[m]
                ps = psB.tile([128, 512], F32, tag="ps")
                for k in range(KD):
                    nc.tensor.matmul(
                        ps[:],
                        lhsT=wqk_sb[k][:, m * 128:(m + 1) * 128],
                        rhs=xT_sb[k][:, n * 512:(n + 1) * 512],
                        start=(k == 0),
                        stop=(k == KD - 1),
                    )
                nc.vector.tensor_copy(dst[:, n * 512:(n + 1) * 512], ps[:])

            for m in range(KD):
                qk_t = qkpool.tile([128, T], BF16, tag=f"qkT{m}")
                qkT_sb[m] = qk_t

            # ---- attention ---------------------------------------------------
            attnT_sb = []
            for p in range(NPAIR):
                at_t = atpool.tile([128, T], BF16, tag=f"attnT{p}")
                attnT_sb.append(at_t)

            def epilogue(p, i, ppsb, dcp, h2):
                """recip -> PE broadcast -> normalize into attnT (deferred one
                group so the PE queue isn't blocked behind the DVE chain)."""
                rt = spool.tile([1, 512], F32, tag="recip")
                nc.vector.reciprocal_approx_fast(rt[:], dcp[:])
                rtr = spool.tile([1, 512], F32R, tag="recipr")
                nc.vector.tensor_copy(rtr[:], rt[:])
                bt = psB.tile([128, 512], F32, tag="ps")
                nc.tensor.matmul(
                    bt[0:64, :],
                    lhsT=ones_sb[0:1, :],
                    rhs=rtr[:],
                    start=True, stop=True,
                )
                nc.vector.tensor_mul(
                    attnT_sb[p][h2 * 64:(h2 + 1) * 64, i * 512:(i + 1) * 512],
                    ppsb[:],
                    bt[0:64, :],
                )

            pending = []
            # pair 0's projection runs up front; pair p+1's 8 chains are
            # injected two per attention i-group of pair p.
            for n in range(IT):
                proj_chain(0, n)
                proj_chain(NPAIR, n)
            for p in range(NPAIR):
                inject = []
                if p + 1 < NPAIR:
                    inject = [(p + 1, n) for n in range(IT)] + [
                        (NPAIR + p + 1, n) for n in range(IT)
                    ]
                kT = qkT_sb[NPAIR + p]
                qT = qkT_sb[p]

                def emit_S(i, j):
                    st = psS.tile([128, 1024], F32, tag="st")
                    nc.tensor.matmul(
                        st[:, 0:512],
                        lhsT=kT[0:64, j * 128:(j + 1) * 128],
                        rhs=qT[0:64, i * 512:(i + 1) * 512],
                        start=True, stop=True,
                    )
                    nc.tensor.matmul(
                        st[:, 512:1024],
                        lhsT=kT[64:128, j * 128:(j + 1) * 128],
                        rhs=qT[64:128, i * 512:(i + 1) * 512],
                        start=True, stop=True,
                    )
                    return st

                for i in range(IT):
                    js = [j for j in range(JT) if needed[j][i]]
                    ppa = psPV.tile([65, 512], F32, tag="ppa")
                    ppb = psPV.tile([65, 512], F32, tag="ppb")
                    st_next = emit_S(i, js[0])
                    for idx, j in enumerate(js):
                        st = st_next
                        et = epool.tile([128, 1024], BF16)
                        nc.scalar.activation(
                            et[:], st[:], mybir.ActivationFunctionType.Exp
                        )
                        # emit the next S pair now: PE produces it while ACT
                        # runs this exp, keeping the exp pipeline fed.
                        if idx + 1 < len(js):
                            st_next = emit_S(i, js[idx + 1])
                        if apply_mask[j][i]:
                            u = U_sb[(j, i)]
                            uap = u[:]
                            urep = dataclasses.replace(
                                uap, ap=[uap.ap[0], [0, 2], uap.ap[1]]
                            )
                            et3 = et[:].rearrange("q (a b) -> q a b", a=2)
                            nc.vector.tensor_mul(et3, et3, urep)
                        first = idx == 0
                        last = idx == len(js) - 1
                        nc.tensor.matmul(
                            ppa[:],
                            lhsT=vp_sb[j][:, 2 * p, :],
                            rhs=et[:, 0:512],
                            start=first, stop=last,
                        )
                        nc.tensor.matmul(
                            ppb[:],
                            lhsT=vp_sb[j][:, 2 * p + 1, :],
                            rhs=et[:, 512:1024],
                            start=first, stop=last,
                        )
                        # inject next pair's projection chains mid-group
                        # where the exp queue is two deep
                        if inject and idx in (2, 7):
                            proj_chain(*inject.pop(0))
                    # free the PV banks fast: copy out^T and denom to SBUF,
                    # defer the normalize chain by one group.
                    for h2, pp in ((0, ppa), (1, ppb)):
                        ppsb = spool.tile([64, 512], F32, tag="ppsb")
                        nc.vector.tensor_copy(ppsb[:], pp[0:64, :])
                        dcp = spool.tile([1, 512], F32, tag="dcp")
                        nc.vector.tensor_copy(dcp[:], pp[64:65, :])
                        pending.append((p, i, ppsb, dcp, h2))
                    while len(pending) > 2:
                        epilogue(*pending.pop(0))
                while inject:
                    proj_chain(*inject.pop(0))
            for args in pending:
                epilogue(*args)

            # ---- output projection ------------------------------------------
            for mt in range(JT):
                for n in range(2):
                    ps = psB.tile([128, 512], F32)
                    for kp in range(NPAIR):
                        nc.tensor.matmul(
                            ps[:],
                            lhsT=attnT_sb[kp][:, mt * 128:(mt + 1) * 128],
                            rhs=wout_sb[kp][:, n * 512:(n + 1) * 512],
                            start=(kp == 0),
                            stop=(kp == NPAIR - 1),
                        )
                    ot = opool.tile([128, 512], BF16, tag="ot")
                    nc.scalar.copy(ot[:], ps[:])
                    nc.gpsimd.dma_start(
                        out[mt * 128:(mt + 1) * 128, n * 512:(n + 1) * 512],
                        ot[:],
                    )

    nc.compile()
    return nc


def kernel(x, position_ids, Wqkv, Wout, _trace=False, _trace_kwargs=None):
    x = np.asarray(x, dtype=np.float32)
    pos = np.asarray(position_ids)
    Wqkv = np.asarray(Wqkv, dtype=np.float32)
    Wout = np.asarray(Wout, dtype=np.float32)

    needed, apply_mask = _schedule(pos)
    key = (needed.tobytes(), apply_mask.tobytes())
    if key not in _prog_cache:
        _prog_cache.clear()
        _prog_cache[key] = _build(needed, apply_mask)
    nc = _prog_cache[key]

    scale = 1.0 / np.sqrt(HD)
    in_maps = []
    xT_b = {}
    for c in range(NC):
        b, g = c // 2, c % 2
        if b not in xT_b:
            xT_b[b] = np.ascontiguousarray(x[b].T).astype(NP_BF16)
        heads = range(g * HPC, (g + 1) * HPC)
        wqk_c = np.concatenate(
            [Wqkv[:, h * HD:(h + 1) * HD] * scale for h in heads]
            + [Wqkv[:, D + h * HD:D + (h + 1) * HD] for h in heads],
            axis=1,
        ).astype(NP_BF16)
        wv_c = np.concatenate(
            [Wqkv[:, 2 * D + h * HD:2 * D + (h + 1) * HD] for h in heads], axis=1
        ).astype(NP_BF16)
        wout_c = np.concatenate(
            [Wout[h * HD:(h + 1) * HD, :] for h in heads], axis=0
        ).astype(NP_BF16)
        pf = pos[b].astype(np.float32)
        in_maps.append({
            "xT": xT_b[b],
            "wqk": np.ascontiguousarray(wqk_c),
            "wv": np.ascontiguousarray(wv_c),
            "wout": np.ascontiguousarray(wout_c),
            "posq": pf.reshape(1, T),
            "posk": np.ascontiguousarray(pf.reshape(JT, 128).T),
        })

    kw = dict(_trace_kwargs or {})
    res = run_bass_kernel_spmd(
        nc, in_maps, core_ids=list(range(NC)), trace=_trace, **kw
    )
    kernel.last_result = res
    out = np.empty((B, T, D), dtype=np.float32)
    for b in range(B):
        out[b] = res.results[2 * b]["out"].astype(np.float32)
        out[b] += res.results[2 * b + 1]["out"].astype(np.float32)
    return out
